# revision 1
# baseline (speedup 1.0000x reference)
"""Bass/Tile Trainium2 kernel for nn_CrossAttentionLayer — v4.

All four matmul stages (QG-projection, V-projection, scores E, A@V) run as
3-term hi/lo fp8 DoubleRow passes: X@Y ~= Xhi@Yhi + Xhi@Ylo + Xlo@Yhi, where
hi = fp8(x), lo = fp8(x - hi).  Each 3-pass fp8 group costs 0.75x the bf16
matmul it replaces (DoubleRow contracts 2 k-tiles per instruction at 0.5
cycles/row) while keeping bf16-level accuracy (residual error ~ delta^2).

Scaling scheme (keeps hi/lo splits out of fp8-e4m3's subnormal range):
  G' = 16 * Wq^T Wk, WvT' = 16 * Wv^T  (host)
  QG' = h1 @ G' = 16*QG;  E' = QG' @ h2^T = 16*E
  P = exp(E' / (16*32) - 1.5)  (shift keeps P < 240 for fp8)
  V' = h2 @ WvT' = 16*V;  out = (P@V') / (16 * P@1)   (shift cancels)
"""

import math
import sys

import numpy as np

sys.path.insert(0, "/opt/trn_rl_repo")

import ml_dtypes

import concourse.bass as bass
import concourse.tile as tile
from concourse import bacc, mybir
from concourse.bass_utils import run_bass_kernel_spmd

BF16 = mybir.dt.bfloat16
F32 = mybir.dt.float32
F8 = mybir.dt.float8e4
DR = mybir.MatmulPerfMode.DoubleRow
ESCALE = 16.0
SHIFT = 1.5

B, N, M, D, HID, OUT = 8, 2048, 2048, 1024, 1024, 1024
N_CORES = 8
P = 128
FREE = 512


def emit_kernel(tc, h1hi, h1lo, h2hi, h2lo, maskT, Ghi, Glo, Wvhi, Wvlo, out):
    nc = tc.nc
    n, m, d, o = N, M, D, OUT
    KC = d // P
    MC = m // P
    NB = n // FREE
    NS = FREE // P
    OB = o // FREE
    rscale = 1.0 / (ESCALE * math.sqrt(HID))

    def mm3(ps, a, b, ksl_of, dr_pairs):
        """Accumulate 3-term hi/lo fp8 DoubleRow product into psum ps."""
        passes = [(a[0], b[0]), (a[0], b[1]), (a[1], b[0])]
        for ip, (ax, bx) in enumerate(passes):
            for kk in range(dr_pairs):
                nc.tensor.matmul(
                    ps,
                    lhsT=ax[:, 2 * kk : 2 * kk + 2, ksl_of[0]],
                    rhs=bx[:, 2 * kk : 2 * kk + 2, ksl_of[1]],
                    start=(ip == 0 and kk == 0),
                    stop=(ip == 2 and kk == dr_pairs - 1),
                    perf_mode=DR,
                )

    with tc.tile_pool(name="persist", bufs=1) as persist:
        h2hi_sb = persist.tile([P, KC, m], F8)
        h2lo_sb = persist.tile([P, KC, m], F8)
        QGThi = persist.tile([P, KC, n], F8)
        QGTlo = persist.tile([P, KC, n], F8)
        Vhi = persist.tile([P, MC, o], F8)    # = fp8 split of 16*V, [m(part), o]
        Vlo = persist.tile([P, MC, o], F8)
        ones2 = persist.tile([P, 2, 1], F8)
        nc.vector.memset(ones2[:], 1.0)
        bias_sb = persist.tile([P, 1], F32)
        nc.vector.memset(bias_sb[:], -SHIFT)

        # ---- phase A: projections (3-pass fp8 DR) ----
        with (
            tc.tile_pool(name="pG", bufs=1) as pG,
            tc.tile_pool(name="pW", bufs=1) as pW,
            tc.tile_pool(name="pH1", bufs=1) as pH1,
            tc.tile_pool(name="psQ", bufs=2, space="PSUM") as psQ,
            tc.tile_pool(name="psV", bufs=2, space="PSUM") as psV,
        ):
            Ghi_sb = pG.tile([P, KC, d], F8)
            Glo_sb = pG.tile([P, KC, d], F8)
            Wvhi_sb = pW.tile([P, KC, o], F8)
            Wvlo_sb = pW.tile([P, KC, o], F8)
            h1hi_sb = pH1.tile([P, KC, n], F8)
            h1lo_sb = pH1.tile([P, KC, n], F8)
            re = lambda t: t.rearrange("(kc p) e -> p kc e", p=P)
            nc.sync.dma_start(Ghi_sb[:], re(Ghi))
            nc.sync.dma_start(Glo_sb[:], re(Glo))
            nc.sync.dma_start(h1hi_sb[:], re(h1hi))
            nc.sync.dma_start(h1lo_sb[:], re(h1lo))
            nc.sync.dma_start(Wvhi_sb[:], re(Wvhi))
            nc.sync.dma_start(Wvlo_sb[:], re(Wvlo))
            nc.sync.dma_start(h2hi_sb[:], re(h2hi))
            nc.sync.dma_start(h2lo_sb[:], re(h2lo))

            # QGT'[d', nb] = sum_dc G'[dc, d']^T . h1T[dc, nb]  (= 16*QG^T)
            for nb in range(NB):
                nsl = slice(nb * FREE, (nb + 1) * FREE)
                for dc2 in range(KC):
                    ps = psQ.tile([P, FREE], F32)
                    mm3(ps[:], (Ghi_sb, Glo_sb), (h1hi_sb, h1lo_sb),
                        (slice(dc2 * P, (dc2 + 1) * P), nsl), KC // 2)
                    nc.scalar.copy(QGThi[:, dc2, nsl], ps[:])
                    nc.vector.scalar_tensor_tensor(
                        QGTlo[:, dc2, nsl], ps[:], 1.0, QGThi[:, dc2, nsl],
                        op0=mybir.AluOpType.mult,
                        op1=mybir.AluOpType.subtract,
                    )

            # V'[mc, :] = sum_dc h2T[dc, mc]^T . WvT'[dc, :]  (= 16*V)
            for mc in range(MC):
                ps = psV.tile([P, o], F32)
                for ob in range(OB):
                    osl = slice(ob * FREE, (ob + 1) * FREE)
                    mm3(ps[:, osl], (h2hi_sb, h2lo_sb), (Wvhi_sb, Wvlo_sb),
                        (slice(mc * P, (mc + 1) * P), osl), KC // 2)
                nc.scalar.copy(Vhi[:, mc, :], ps[:])
                nc.vector.scalar_tensor_tensor(
                    Vlo[:, mc, :], ps[:], 1.0, Vhi[:, mc, :],
                    op0=mybir.AluOpType.mult,
                    op1=mybir.AluOpType.subtract,
                )

        # ---- phase B: E^T -> exp*mask -> split -> A^T V ----
        with (
            tc.tile_pool(name="psE", bufs=2, space="PSUM") as psE,
            tc.tile_pool(name="psAV", bufs=2, space="PSUM") as psAV,
            tc.tile_pool(name="psDen", bufs=2, space="PSUM") as psDen,
            tc.tile_pool(name="maskp", bufs=2) as maskp,
            tc.tile_pool(name="ptstage", bufs=3) as ptstage,
            tc.tile_pool(name="pthi", bufs=2) as pthi,
            tc.tile_pool(name="ptlo", bufs=2) as ptlo,
            tc.tile_pool(name="outp", bufs=3) as outp,
            tc.tile_pool(name="smalls", bufs=4) as smalls,
        ):
            for nb in range(NB):
                nsl = slice(nb * FREE, (nb + 1) * FREE)
                mT = maskp.tile([P, MC, FREE], BF16)
                for mc in range(MC):
                    nc.sync.dma_start(
                        mT[:, mc, :],
                        maskT.rearrange("(mc p) e -> p mc e", p=P)[:, mc, nsl],
                    )

                PThi = pthi.tile([P, MC, FREE], F8)
                PTlo = ptlo.tile([P, MC, FREE], F8)
                for mc in range(MC):
                    ps = psE.tile([P, FREE], F32)
                    mm3(ps[:], (h2hi_sb, h2lo_sb), (QGThi, QGTlo),
                        (slice(mc * P, (mc + 1) * P), nsl), KC // 2)
                    # P = exp(E'/(16*32) - 1.5) * mask, staged bf16 then split
                    stage = ptstage.tile([P, FREE], BF16)
                    nc.scalar.activation(
                        stage[:], ps[:], mybir.ActivationFunctionType.Exp,
                        scale=rscale, bias=bias_sb[:],
                    )
                    nc.vector.tensor_mul(stage[:], stage[:], mT[:, mc, :])
                    nc.scalar.copy(PThi[:, mc, :], stage[:])
                    nc.vector.scalar_tensor_tensor(
                        PTlo[:, mc, :], stage[:], 1.0, PThi[:, mc, :],
                        op0=mybir.AluOpType.mult,
                        op1=mybir.AluOpType.subtract,
                    )

                # out[ns] = (PT^T @ V') / (16 * PT^T @ 1)
                for ns in range(NS):
                    po = psAV.tile([P, o], F32)
                    pden = psDen.tile([P, 1], F32)
                    av_passes = [(PThi, Vhi), (PThi, Vlo), (PTlo, Vhi)]
                    for ip, (px, vx) in enumerate(av_passes):
                        for mcc in range(MC // 2):
                            lhs = px[:, 2 * mcc : 2 * mcc + 2,
                                     ns * P : (ns + 1) * P]
                            for ob in range(OB):
                                nc.tensor.matmul(
                                    po[:, ob * FREE : (ob + 1) * FREE],
                                    lhsT=lhs,
                                    rhs=vx[:, 2 * mcc : 2 * mcc + 2,
                                           ob * FREE : (ob + 1) * FREE],
                                    start=(ip == 0 and mcc == 0),
                                    stop=(ip == 2 and mcc == MC // 2 - 1),
                                    perf_mode=DR,
                                )
                            if ip < 2:  # den = sum(PThi) + sum(PTlo)
                                dlhs = lhs if ip == 0 else PTlo[
                                    :, 2 * mcc : 2 * mcc + 2, ns * P : (ns + 1) * P]
                                nc.tensor.matmul(
                                    pden[:], lhsT=dlhs, rhs=ones2[:],
                                    start=(ip == 0 and mcc == 0),
                                    stop=(ip == 1 and mcc == MC // 2 - 1),
                                    perf_mode=DR,
                                )
                    rden = smalls.tile([P, 1], F32)
                    nc.vector.reciprocal(rden[:], pden[:])
                    nc.vector.tensor_scalar_mul(rden[:], rden[:], 1.0 / ESCALE)
                    ob_sb = outp.tile([P, o], BF16)
                    nc.scalar.activation(
                        ob_sb[:], po[:], mybir.ActivationFunctionType.Copy,
                        scale=rden[:],
                    )
                    r0 = nb * FREE + ns * P
                    nc.sync.dma_start(out[r0 : r0 + P, :], ob_sb[:])


def build_nc(n_cores=N_CORES, reps=1):
    nc = bacc.Bacc(
        "TRN2",
        target_bir_lowering=False,
        debug=False,
        enable_asserts=False,
        num_devices=n_cores,
    )
    t = lambda nm, shp, dt: nc.dram_tensor(nm, shp, dt, kind="ExternalInput").ap()
    h1hi = t("h1hi", [D, N], F8)
    h1lo = t("h1lo", [D, N], F8)
    h2hi = t("h2hi", [D, M], F8)
    h2lo = t("h2lo", [D, M], F8)
    maskT = t("maskT", [M, N], BF16)
    Ghi = t("Ghi", [D, D], F8)
    Glo = t("Glo", [D, D], F8)
    Wvhi = t("Wvhi", [D, OUT], F8)
    Wvlo = t("Wvlo", [D, OUT], F8)
    out = nc.dram_tensor("out", [N, OUT], BF16, kind="ExternalOutput").ap()
    with tile.TileContext(nc) as tc:
        for _ in range(reps):
            emit_kernel(tc, h1hi, h1lo, h2hi, h2lo, maskT, Ghi, Glo,
                        Wvhi, Wvlo, out)
    nc.compile()
    return nc


def _to_bf16(x_f32):
    x = np.ascontiguousarray(x_f32, dtype=np.float32)
    u = x.view(np.uint32)
    r = ((u >> np.uint32(16)) & np.uint32(1)) + np.uint32(0x7FFF)
    return ((u + r) >> np.uint32(16)).astype(np.uint16).view(ml_dtypes.bfloat16)


def _split8(x_f32):
    hi = x_f32.astype(ml_dtypes.float8_e4m3)
    lo = (x_f32 - hi.astype(np.float32)).astype(ml_dtypes.float8_e4m3)
    return hi, lo


def prep_inputs(h1, h2, mask, Wq, Wk, Wv):
    S = np.float32(ESCALE)
    G = _to_bf16(
        (Wq.astype(np.float32, copy=False).T @ Wk.astype(np.float32, copy=False)) * S
    ).astype(np.float32)
    Ghi, Glo = _split8(G)
    WvT = _to_bf16(np.ascontiguousarray(Wv.astype(np.float32, copy=False).T) * S
                   ).astype(np.float32)
    Wvhi, Wvlo = _split8(WvT)
    h1T = _to_bf16(np.ascontiguousarray(
        np.asarray(h1, np.float32).transpose(0, 2, 1))).astype(np.float32)
    h1hi, h1lo = _split8(h1T)
    h2T = _to_bf16(np.ascontiguousarray(
        np.asarray(h2, np.float32).transpose(0, 2, 1))).astype(np.float32)
    h2hi, h2lo = _split8(h2T)
    mT = (np.ascontiguousarray(np.asarray(mask).transpose(0, 2, 1)).astype(np.uint16)
          * np.uint16(0x3F80)).view(ml_dtypes.bfloat16)
    return [
        {"h1hi": h1hi[b], "h1lo": h1lo[b], "h2hi": h2hi[b], "h2lo": h2lo[b],
         "maskT": mT[b], "Ghi": Ghi, "Glo": Glo, "Wvhi": Wvhi, "Wvlo": Wvlo}
        for b in range(B)
    ]


_NC_CACHE = {}


def get_nc():
    if "nc" not in _NC_CACHE:
        _NC_CACHE["nc"] = build_nc()
    return _NC_CACHE["nc"]


def run(in_maps, trace=False):
    return run_bass_kernel_spmd(get_nc(), in_maps, list(range(N_CORES)), trace=trace)


def kernel(h1, h2, mask, Wq, Wk, Wv):
    in_maps = prep_inputs(h1, h2, mask, Wq, Wk, Wv)
    res = run(in_maps)
    return np.stack(
        [res.results[b]["out"].astype(np.float32) for b in range(B)], axis=0)



# revision 2
# speedup vs baseline: 1.6025x; 1.6025x over previous
"""Bass/Tile Trainium2 kernel for nn_CrossAttentionLayer — v5.

All four matmul stages run as single-pass fp16 matmuls (1 cycle/row on the
PE array — ~1.5x cheaper than the previous 3-term hi/lo fp8 DoubleRow
scheme, and ~10x more accurate: fp16's 10-bit mantissa keeps each stage's
relative error ~5e-4 vs ~3.6e-2 for fp8-hi-only, which fails the 2e-2 gate).

Math (host precomputes G = Wq^T Wk, folding two projections into one):
  G' = 16 * Wq^T Wk, WvT' = 16 * Wv^T              (host, fp16)
  QG' = h1 @ G'      (= 16*QG)                      [N, D]
  E'  = QG' @ h2^T   (= 16*E), computed as E'^T     [M, N] tiles
  P   = fp16(exp(E' / (16*32))) * mask              [M, N] tiles
  V'  = h2 @ WvT'    (= 16*V)                       [M, OUT]
  out = (P^T @ V') / (16 * sum_m P)
The softmax denominator comes from S[p, n] = sum_mc P[mc*128+p, n]
(accumulated on DVE during the exp/mask pass), then one ones-matmul per
128-row output tile reduces S across partitions.
"""

import math
import sys

import numpy as np

sys.path.insert(0, "/opt/trn_rl_repo")

import concourse.bass as bass
import concourse.tile as tile
from concourse import bacc, mybir
from concourse.bass_utils import run_bass_kernel_spmd

F16 = mybir.dt.float16
F32 = mybir.dt.float32
ESCALE = 16.0

B, N, M, D, HID, OUT = 8, 2048, 2048, 1024, 1024, 1024
N_CORES = 8
P = 128
FREE = 512


def emit_kernel(tc, h1T, h2T, maskT, G, WvT, out):
    nc = tc.nc
    KC = D // P      # 8 contraction chunks of 128
    MC = M // P      # 16
    NB = N // FREE   # 4
    NS = FREE // P   # 4
    OB = OUT // FREE  # 2
    rscale = 1.0 / (ESCALE * math.sqrt(HID))

    with tc.tile_pool(name="persist", bufs=1) as persist:
        h2sb = persist.tile([P, KC, M], F16)   # h2^T as [d(part), m]
        QGT = persist.tile([P, KC, N], F16)    # QG'^T as [d(part), n]
        V = persist.tile([P, MC, OUT], F16)    # V' as [m(part), o]
        ones = persist.tile([P, 1], F16)
        nc.vector.memset(ones[:], 1.0)

        re = lambda t: t.rearrange("(kc p) e -> p kc e", p=P)

        # ---- phase A: projections ----
        with (
            tc.tile_pool(name="pA", bufs=1) as pA,
            tc.tile_pool(name="psQ", bufs=2, space="PSUM") as psQ,
            tc.tile_pool(name="psV", bufs=2, space="PSUM") as psV,
        ):
            Gsb = pA.tile([P, KC, HID], F16)
            h1sb = pA.tile([P, KC, N], F16)
            Wvsb = pA.tile([P, KC, OUT], F16)
            nc.sync.dma_start(Gsb[:], re(G))
            for nb in range(NB):
                nsl = slice(nb * FREE, (nb + 1) * FREE)
                nc.sync.dma_start(h1sb[:, :, nsl], re(h1T)[:, :, nsl])
            nc.sync.dma_start(h2sb[:], re(h2T))
            nc.sync.dma_start(Wvsb[:], re(WvT))

            # QG'^T[d', n] = sum_kc G'[kc, d']^T . h1T[kc, n]
            for nb in range(NB):
                nsl = slice(nb * FREE, (nb + 1) * FREE)
                for dc in range(KC):
                    ps = psQ.tile([P, FREE], F32)
                    for kc in range(KC):
                        nc.tensor.matmul(
                            ps[:],
                            lhsT=Gsb[:, kc, dc * P : (dc + 1) * P],
                            rhs=h1sb[:, kc, nsl],
                            start=(kc == 0),
                            stop=(kc == KC - 1),
                        )
                    nc.vector.tensor_scalar_mul(QGT[:, dc, nsl], ps[:], 1.0)

            # V'[m, o] = sum_kc h2T[kc, m]^T . WvT'[kc, o]
            for mc in range(MC):
                ps = psV.tile([P, OUT], F32)
                for kc in range(KC):
                    for ob in range(OB):
                        osl = slice(ob * FREE, (ob + 1) * FREE)
                        nc.tensor.matmul(
                            ps[:, osl],
                            lhsT=h2sb[:, kc, mc * P : (mc + 1) * P],
                            rhs=Wvsb[:, kc, osl],
                            start=(kc == 0),
                            stop=(kc == KC - 1),
                        )
                nc.scalar.copy(V[:, mc, :], ps[:])

        # ---- phase B: E^T -> exp*mask -> P^T V ----
        with (
            tc.tile_pool(name="psE", bufs=2, space="PSUM") as psE,
            tc.tile_pool(name="psAV", bufs=2, space="PSUM") as psAV,
            tc.tile_pool(name="psDen", bufs=2, space="PSUM") as psDen,
            tc.tile_pool(name="maskp", bufs=2) as maskp,
            tc.tile_pool(name="ptp", bufs=2) as ptp,
            tc.tile_pool(name="sp", bufs=2) as sp,
            tc.tile_pool(name="stg", bufs=3) as stg,
            tc.tile_pool(name="outp", bufs=3) as outp,
            tc.tile_pool(name="smalls", bufs=4) as smalls,
        ):
            mre = maskT.rearrange("(mc p) e -> p mc e", p=P)
            for nb in range(NB):
                nsl = slice(nb * FREE, (nb + 1) * FREE)
                mT = maskp.tile([P, MC, FREE], F16)
                for mc in range(MC):
                    nc.sync.dma_start(mT[:, mc, :], mre[:, mc, nsl])

                PT = ptp.tile([P, MC, FREE], F16)
                S = sp.tile([P, FREE], F16)
                for mc in range(MC):
                    ps = psE.tile([P, FREE], F32)
                    for kc in range(KC):
                        nc.tensor.matmul(
                            ps[:],
                            lhsT=h2sb[:, kc, mc * P : (mc + 1) * P],
                            rhs=QGT[:, kc, nsl],
                            start=(kc == 0),
                            stop=(kc == KC - 1),
                        )
                    stage = stg.tile([P, FREE], F16)
                    nc.scalar.activation(
                        stage[:], ps[:], mybir.ActivationFunctionType.Exp,
                        scale=rscale,
                    )
                    nc.vector.tensor_mul(PT[:, mc, :], stage[:], mT[:, mc, :])
                    if mc == 0:
                        nc.vector.tensor_scalar_mul(S[:], PT[:, 0, :], 1.0)
                    else:
                        nc.vector.scalar_tensor_tensor(
                            S[:], PT[:, mc, :], 1.0, S[:],
                            op0=mybir.AluOpType.mult,
                            op1=mybir.AluOpType.add,
                        )

                # out[ns] = (PT^T @ V') / (16 * ones^T S)
                for ns in range(NS):
                    po = psAV.tile([P, OUT], F32)
                    pden = psDen.tile([P, 1], F32)
                    psl = slice(ns * P, (ns + 1) * P)
                    for mc in range(MC):
                        for ob in range(OB):
                            osl = slice(ob * FREE, (ob + 1) * FREE)
                            nc.tensor.matmul(
                                po[:, osl],
                                lhsT=PT[:, mc, psl],
                                rhs=V[:, mc, osl],
                                start=(mc == 0),
                                stop=(mc == MC - 1),
                            )
                    nc.tensor.matmul(pden[:], lhsT=S[:, psl], rhs=ones[:],
                                     start=True, stop=True)
                    rden = smalls.tile([P, 1], F32)
                    nc.vector.reciprocal(rden[:], pden[:])
                    nc.vector.tensor_scalar_mul(rden[:], rden[:], 1.0 / ESCALE)
                    ob_sb = outp.tile([P, OUT], F16)
                    nc.scalar.activation(
                        ob_sb[:], po[:], mybir.ActivationFunctionType.Copy,
                        scale=rden[:],
                    )
                    r0 = nb * FREE + ns * P
                    nc.sync.dma_start(out[r0 : r0 + P, :], ob_sb[:])


def build_nc(n_cores=N_CORES, reps=1):
    nc = bacc.Bacc(
        "TRN2",
        target_bir_lowering=False,
        debug=False,
        enable_asserts=False,
        num_devices=n_cores,
    )
    t = lambda nm, shp, dt: nc.dram_tensor(nm, shp, dt, kind="ExternalInput").ap()
    h1T = t("h1T", [D, N], F16)
    h2T = t("h2T", [D, M], F16)
    maskT = t("maskT", [M, N], F16)
    G = t("G", [D, HID], F16)
    WvT = t("WvT", [D, OUT], F16)
    out = nc.dram_tensor("out", [N, OUT], F16, kind="ExternalOutput").ap()
    with tile.TileContext(nc) as tc:
        for _ in range(reps):
            emit_kernel(tc, h1T, h2T, maskT, G, WvT, out)
    nc.compile()
    return nc


def prep_inputs(h1, h2, mask, Wq, Wk, Wv):
    S = np.float32(ESCALE)
    G = ((Wq.astype(np.float32, copy=False).T @
          Wk.astype(np.float32, copy=False)) * S).astype(np.float16)
    WvT = (np.ascontiguousarray(Wv.astype(np.float32, copy=False).T) * S
           ).astype(np.float16)
    h1T = np.ascontiguousarray(
        np.asarray(h1, np.float32).transpose(0, 2, 1)).astype(np.float16)
    h2T = np.ascontiguousarray(
        np.asarray(h2, np.float32).transpose(0, 2, 1)).astype(np.float16)
    mT = np.ascontiguousarray(
        np.asarray(mask).transpose(0, 2, 1)).astype(np.float16)
    return [
        {"h1T": h1T[b], "h2T": h2T[b], "maskT": mT[b], "G": G, "WvT": WvT}
        for b in range(B)
    ]


_NC_CACHE = {}


def get_nc():
    if "nc" not in _NC_CACHE:
        _NC_CACHE["nc"] = build_nc()
    return _NC_CACHE["nc"]


def run(in_maps, trace=False):
    return run_bass_kernel_spmd(get_nc(), in_maps, list(range(N_CORES)), trace=trace)


def kernel(h1, h2, mask, Wq, Wk, Wv):
    in_maps = prep_inputs(h1, h2, mask, Wq, Wk, Wv)
    res = run(in_maps)
    return np.stack(
        [res.results[b]["out"].astype(np.float32) for b in range(B)], axis=0)


# revision 7
# speedup vs baseline: 1.6110x; 1.0053x over previous
"""Bass/Tile Trainium2 kernel for nn_CrossAttentionLayer — v5.

All four matmul stages run as single-pass fp16 matmuls (1 cycle/row on the
PE array — ~1.5x cheaper than the previous 3-term hi/lo fp8 DoubleRow
scheme, and ~10x more accurate: fp16's 10-bit mantissa keeps each stage's
relative error ~5e-4 vs ~3.6e-2 for fp8-hi-only, which fails the 2e-2 gate).

Math (host precomputes G = Wq^T Wk, folding two projections into one):
  G' = 16 * Wq^T Wk, WvT' = 16 * Wv^T              (host, fp16)
  QG' = h1 @ G'      (= 16*QG)                      [N, D]
  E'  = QG' @ h2^T   (= 16*E), computed as E'^T     [M, N] tiles
  P   = fp16(exp(E' / (16*32))) * mask              [M, N] tiles
  V'  = h2 @ WvT'    (= 16*V)                       [M, OUT]
  out = (P^T @ V') / (16 * sum_m P)
The softmax denominator comes from S[p, n] = sum_mc P[mc*128+p, n]
(accumulated on DVE during the exp/mask pass), then one ones-matmul per
128-row output tile reduces S across partitions.
"""

import math
import sys

import numpy as np

sys.path.insert(0, "/opt/trn_rl_repo")

import concourse.bass as bass
import concourse.tile as tile
from concourse import bacc, mybir
from concourse.bass_utils import run_bass_kernel_spmd

F16 = mybir.dt.float16
F32 = mybir.dt.float32
ESCALE = 16.0

B, N, M, D, HID, OUT = 8, 2048, 2048, 1024, 1024, 1024
N_CORES = 8
P = 128
FREE = 512


def emit_kernel(tc, h1T, h2T, maskT, G, WvT, out):
    nc = tc.nc
    KC = D // P      # 8 contraction chunks of 128
    MC = M // P      # 16
    NB = N // FREE   # 4
    NS = FREE // P   # 4
    OB = OUT // FREE  # 2
    rscale = 1.0 / (ESCALE * math.sqrt(HID))

    with tc.tile_pool(name="persist", bufs=1) as persist:
        h2sb = persist.tile([P, KC, M], F16)   # h2^T as [d(part), m]
        QGT = persist.tile([P, KC, N], F16)    # QG'^T as [d(part), n]
        V = persist.tile([P, MC, OUT], F16)    # V' as [m(part), o]
        ones = persist.tile([P, 1], F16)
        nc.vector.memset(ones[:], 1.0)

        re = lambda t: t.rearrange("(kc p) e -> p kc e", p=P)

        # ---- phase A: projections ----
        with (
            tc.tile_pool(name="pA", bufs=1) as pA,
            tc.tile_pool(name="psQ", bufs=2, space="PSUM") as psQ,
            tc.tile_pool(name="psV", bufs=2, space="PSUM") as psV,
        ):
            Gsb = pA.tile([P, KC, HID], F16)
            h1sb = pA.tile([P, KC, N], F16)
            Wvsb = pA.tile([P, KC, OUT], F16)
            # First QG tile (nb=0, dc=0) needs only G[:, :, 0:128] and
            # h1[:, :, 0:512] — order DMAs so the PE starts ~3.5us in.
            nc.sync.dma_start(Gsb[:, :, 0:P], re(G)[:, :, 0:P])
            nc.sync.dma_start(h1sb[:, :, 0:FREE], re(h1T)[:, :, 0:FREE])
            for dc in range(1, KC):
                dsl = slice(dc * P, (dc + 1) * P)
                nc.sync.dma_start(Gsb[:, :, dsl], re(G)[:, :, dsl])
            for nb in range(1, NB):
                nsl = slice(nb * FREE, (nb + 1) * FREE)
                nc.sync.dma_start(h1sb[:, :, nsl], re(h1T)[:, :, nsl])
            nc.sync.dma_start(h2sb[:], re(h2T))
            nc.sync.dma_start(Wvsb[:], re(WvT))

            # QG'^T[d', n] = sum_kc G'[kc, d']^T . h1T[kc, n]
            for nb in range(NB):
                nsl = slice(nb * FREE, (nb + 1) * FREE)
                for dc in range(KC):
                    ps = psQ.tile([P, FREE], F32)
                    for kc in range(KC):
                        nc.tensor.matmul(
                            ps[:],
                            lhsT=Gsb[:, kc, dc * P : (dc + 1) * P],
                            rhs=h1sb[:, kc, nsl],
                            start=(kc == 0),
                            stop=(kc == KC - 1),
                        )
                    nc.vector.tensor_scalar_mul(QGT[:, dc, nsl], ps[:], 1.0)

            # V'[m, o] = sum_kc h2T[kc, m]^T . WvT'[kc, o]
            for mc in range(MC):
                ps = psV.tile([P, OUT], F32)
                for kc in range(KC):
                    for ob in range(OB):
                        osl = slice(ob * FREE, (ob + 1) * FREE)
                        nc.tensor.matmul(
                            ps[:, osl],
                            lhsT=h2sb[:, kc, mc * P : (mc + 1) * P],
                            rhs=Wvsb[:, kc, osl],
                            start=(kc == 0),
                            stop=(kc == KC - 1),
                        )
                nc.scalar.copy(V[:, mc, :], ps[:])

        # ---- phase B: E^T -> exp*mask -> P^T V ----
        with (
            tc.tile_pool(name="psE", bufs=2, space="PSUM") as psE,
            tc.tile_pool(name="psAV", bufs=2, space="PSUM") as psAV,
            tc.tile_pool(name="maskp", bufs=2) as maskp,
            tc.tile_pool(name="ptp", bufs=2) as ptp,
            tc.tile_pool(name="sp", bufs=2) as sp,
            tc.tile_pool(name="stg", bufs=3) as stg,
            tc.tile_pool(name="outp", bufs=3) as outp,
            tc.tile_pool(name="smalls", bufs=4) as smalls,
        ):
            mre = maskT.rearrange("(mc p) e -> p mc e", p=P)
            # nb-blocks processed in pairs so each E lhsT load serves 2
            # matmuls (halves the LDWEIGHTS count of the biggest stage).
            for pb in range(NB // 2):
                nbs = (2 * pb, 2 * pb + 1)
                nsls = [slice(nb * FREE, (nb + 1) * FREE) for nb in nbs]
                mTs, PTs, Ss = [], [], []
                for h, nb in enumerate(nbs):
                    mT = maskp.tile([P, MC, FREE], F16)
                    for mc in range(MC):
                        nc.sync.dma_start(mT[:, mc, :], mre[:, mc, nsls[h]])
                    mTs.append(mT)
                    PTs.append(ptp.tile([P, MC, FREE], F16, name=f"PT{h}"))
                    Ss.append(sp.tile([P, FREE], F16, name=f"S{h}"))

                for mc in range(MC):
                    msl = slice(mc * P, (mc + 1) * P)
                    psEp = psE.tile([P, 2, FREE], F32, tag="eps")
                    pss = [psEp[:, h, :] for h in range(2)]
                    for kc in range(KC):
                        for h in range(2):
                            nc.tensor.matmul(
                                pss[h],
                                lhsT=h2sb[:, kc, msl],
                                rhs=QGT[:, kc, nsls[h]],
                                start=(kc == 0),
                                stop=(kc == KC - 1),
                            )
                    for h in range(2):
                        stage = stg.tile([P, FREE], F16)
                        nc.scalar.activation(
                            stage[:], pss[h],
                            mybir.ActivationFunctionType.Exp, scale=rscale,
                        )
                        nc.vector.tensor_mul(
                            PTs[h][:, mc, :], stage[:], mTs[h][:, mc, :])
                        if mc == 0:
                            nc.vector.tensor_scalar_mul(
                                Ss[h][:], PTs[h][:, 0, :], 1.0)
                        else:
                            nc.vector.scalar_tensor_tensor(
                                Ss[h][:], PTs[h][:, mc, :], 1.0, Ss[h][:],
                                op0=mybir.AluOpType.mult,
                                op1=mybir.AluOpType.add,
                            )

                # out[ns] = (PT^T @ V') / (16 * ones^T S)
                for h, nb in enumerate(nbs):
                    PT, S = PTs[h], Ss[h]
                    for ns in range(NS):
                        po = psAV.tile([P, OUT], F32)
                        pden = psE.tile([P, 1], F32, name="pden", tag="eps")
                        psl = slice(ns * P, (ns + 1) * P)
                        for mc in range(MC):
                            for ob in range(OB):
                                osl = slice(ob * FREE, (ob + 1) * FREE)
                                nc.tensor.matmul(
                                    po[:, osl],
                                    lhsT=PT[:, mc, psl],
                                    rhs=V[:, mc, osl],
                                    start=(mc == 0),
                                    stop=(mc == MC - 1),
                                )
                        nc.tensor.matmul(pden[:], lhsT=S[:, psl], rhs=ones[:],
                                         start=True, stop=True)
                        rden = smalls.tile([P, 1], F32)
                        nc.vector.reciprocal(rden[:], pden[:])
                        nc.vector.tensor_scalar_mul(
                            rden[:], rden[:], 1.0 / ESCALE)
                        ob_sb = outp.tile([P, OUT], F16)
                        nc.scalar.activation(
                            ob_sb[:], po[:], mybir.ActivationFunctionType.Copy,
                            scale=rden[:],
                        )
                        r0 = nb * FREE + ns * P
                        nc.sync.dma_start(out[r0 : r0 + P, :], ob_sb[:])


def build_nc(n_cores=N_CORES, reps=1):
    nc = bacc.Bacc(
        "TRN2",
        target_bir_lowering=False,
        debug=False,
        enable_asserts=False,
        num_devices=n_cores,
    )
    t = lambda nm, shp, dt: nc.dram_tensor(nm, shp, dt, kind="ExternalInput").ap()
    h1T = t("h1T", [D, N], F16)
    h2T = t("h2T", [D, M], F16)
    maskT = t("maskT", [M, N], F16)
    G = t("G", [D, HID], F16)
    WvT = t("WvT", [D, OUT], F16)
    out = nc.dram_tensor("out", [N, OUT], F16, kind="ExternalOutput").ap()
    with tile.TileContext(nc) as tc:
        for _ in range(reps):
            emit_kernel(tc, h1T, h2T, maskT, G, WvT, out)
    nc.compile()
    return nc


def prep_inputs(h1, h2, mask, Wq, Wk, Wv):
    S = np.float32(ESCALE)
    G = ((Wq.astype(np.float32, copy=False).T @
          Wk.astype(np.float32, copy=False)) * S).astype(np.float16)
    WvT = (np.ascontiguousarray(Wv.astype(np.float32, copy=False).T) * S
           ).astype(np.float16)
    h1T = np.ascontiguousarray(
        np.asarray(h1, np.float32).transpose(0, 2, 1)).astype(np.float16)
    h2T = np.ascontiguousarray(
        np.asarray(h2, np.float32).transpose(0, 2, 1)).astype(np.float16)
    mT = np.ascontiguousarray(
        np.asarray(mask).transpose(0, 2, 1)).astype(np.float16)
    return [
        {"h1T": h1T[b], "h2T": h2T[b], "maskT": mT[b], "G": G, "WvT": WvT}
        for b in range(B)
    ]


_NC_CACHE = {}


def get_nc():
    if "nc" not in _NC_CACHE:
        _NC_CACHE["nc"] = build_nc()
    return _NC_CACHE["nc"]


def run(in_maps, trace=False):
    return run_bass_kernel_spmd(get_nc(), in_maps, list(range(N_CORES)), trace=trace)


def kernel(h1, h2, mask, Wq, Wk, Wv):
    in_maps = prep_inputs(h1, h2, mask, Wq, Wk, Wv)
    res = run(in_maps)
    return np.stack(
        [res.results[b]["out"].astype(np.float32) for b in range(B)], axis=0)
